# revision 15
# baseline (speedup 1.0000x reference)
"""GCN layer on 8 trn2 NeuronCores.

out = tanh( (D^-1/2 (adj+I) D^-1/2) @ H @ W.T + b ), N=8192, nin=nout=512.

Associativity + normalization folding: with d = deg^-0.5,
  out = tanh( S''^T @ HsW + b )  where
  S''[k, m] = d_m * (adj + I)[m, k]   (fully-normalized adjacency, on host)
  HsW[k, :] = d_k * (H @ W.T)[k, :]   (W folded into H on host: one small
                                       4.3-GFLOP BLAS gemm)
so the device runs a SINGLE big gemm (8192x1024x512 per core, bf16) plus a
fused bias+tanh activation per PSUM bank. No second gemm, no transposes, no
PSUM->SBUF copies.

Sharding: output rows (and adj rows) split across 8 cores, 1024 rows each.

Device per core:
  OutT[nout, m] = sum_k HsW[k, nout] * S''[k, m]
    - stationary lhsT = HsW k-chunk [128, 128] (reused for both 512-col
      m-halves), moving rhs = packed S'' strip [128, 512].
    - 4 nout-chunks x 2 m-halves accumulate the full 64-k-tile contraction
      in all 8 PSUM banks simultaneously.
  res = tanh(acc + b_chunk)  (scalar engine, per-partition bias = b slice)
Output lands transposed ([nout, m] blocks); the host transposes it back.

All SWDGE DMAs drain FIFO on one logical queue, so issue order is arrival
order: HsW slices are interleaved with the S strip chunks in exactly the
order the k-loop consumes them, and the first chunks are small to cut
startup latency (first matmul needs only 0.75 MB). The last chunk runs
bank-major so banks stop staggered and the tanh+store tail overlaps the
remaining matmuls. Output stores alternate HWDGE/SWDGE rings.
"""

import sys

sys.path.insert(0, "/opt/trn_rl_repo")

import numpy as np
import ml_dtypes

from concourse import bass, bacc, tile, mybir
from concourse.bass_utils import run_bass_kernel_spmd

N = 8192
NIN = 512
NOUT = 512
NC = 8
RB = N // NC  # 1024 rows per core
KT = N // 128  # 64 k-tiles
CHUNKS = [2, 2, 4] + [8] * 7  # k-tiles per S strip chunk (sum = 64)
F32 = mybir.dt.float32
BF16 = mybir.dt.bfloat16
NPBF = ml_dtypes.bfloat16

_CACHED_NC = None


def _build():
    nc = bacc.Bacc(None, target_bir_lowering=False)

    # Per-core inputs (packed layouts, see kernel() glue)
    S = nc.dram_tensor("S", [128, KT * RB], BF16, kind="ExternalInput")
    HWP = nc.dram_tensor("HWP", [128, KT * NOUT], BF16, kind="ExternalInput")
    Bt = nc.dram_tensor("Bt", [128, 4], F32, kind="ExternalInput")
    # Output transposed: col block (c*2+mb)*512 holds OutT[c-chunk, mb-half]
    Out = nc.dram_tensor("out", [128, 8 * 512], BF16, kind="ExternalOutput")

    with tile.TileContext(nc) as tc:
        with (
            tc.tile_pool(name="persist", bufs=1) as persist,
            tc.tile_pool(name="strip", bufs=6) as striper,
            tc.tile_pool(name="res", bufs=4) as resp,
            tc.tile_pool(name="acc", bufs=2, space=bass.MemorySpace.PSUM) as pacc,
        ):
            # HsW resident: partition p, col kt*512+q holds HsW[kt*128+p, q]
            hw_big = persist.tile([128, KT * NOUT], BF16)
            b_t = persist.tile([128, 4], F32)

            # Both m-halves accumulate across the whole k loop: 8 banks.
            acc0 = pacc.tile([128, 4 * 512], F32, tag="acc")
            acc1 = pacc.tile([128, 4 * 512], F32, tag="acc")
            accs = (acc0, acc1)

            # HAM warm-up: short dummy matmuls on memset scratch keep the PE
            # busy through its ~3.4us activity window while the first strips
            # DMA, so the real gemm starts at 2.4 GHz instead of 1.2. N=128
            # keeps them cheap (~107ns cold each) so the FIFO clears right as
            # the first data lands; the real start=True clears the bank.
            scratch = persist.tile([128, 128], BF16)
            nc.vector.memset(scratch[:], 0.0)
            for _ in range(40):
                nc.tensor.matmul(
                    acc0[:, 0:128], scratch[:], scratch[:], start=True, stop=True
                )

            def mm(kt, c, mb, strip, ktl):
                nc.tensor.matmul(
                    accs[mb][:, c * 512 : (c + 1) * 512],
                    hw_big[:, kt * NOUT + c * 128 : kt * NOUT + (c + 1) * 128],
                    strip[:, ktl * RB + mb * 512 : ktl * RB + (mb + 1) * 512],
                    start=(kt == 0),
                    stop=(kt == KT - 1),
                )

            kt0 = 0
            for ci, cn in enumerate(CHUNKS):
                # Interleave the HsW slice for this k-range ahead of its
                # strip (single SWDGE FIFO: issue order == arrival order).
                hsl = slice(kt0 * NOUT, (kt0 + cn) * NOUT)
                nc.sync.dma_start(hw_big[:, hsl], HWP[:, hsl])
                strip = striper.tile([128, 8 * RB], BF16, tag="strip")
                ssl = slice(kt0 * RB, (kt0 + cn) * RB)
                nc.gpsimd.dma_start(strip[:, 0 : cn * RB], S[:, ssl])
                if ci == 1:
                    nc.sync.dma_start(b_t[:], Bt[:, :])
                last = ci == len(CHUNKS) - 1
                if not last:
                    for ktl in range(cn):
                        for c in range(4):
                            for mb in range(2):
                                mm(kt0 + ktl, c, mb, strip, ktl)
                else:
                    # Bank-major: each bank stops staggered so its tanh +
                    # store overlaps the remaining banks' matmuls.
                    for c in range(4):
                        for mb in range(2):
                            for ktl in range(cn):
                                mm(kt0 + ktl, c, mb, strip, ktl)
                            res = resp.tile([128, 512], BF16, tag="res")
                            nc.scalar.activation(
                                res[:],
                                accs[mb][:, c * 512 : (c + 1) * 512],
                                mybir.ActivationFunctionType.Tanh,
                                bias=b_t[:, c : c + 1],
                            )
                            blk = (c * 2 + mb) * 512
                            eng = nc.gpsimd if (c * 2 + mb) % 2 == 0 else nc.sync
                            eng.dma_start(Out[:, blk : blk + 512], res[:])
                kt0 += cn

    nc.compile()
    return nc


def kernel(H, adj_matrix, W, b):
    global _CACHED_NC
    H = np.ascontiguousarray(np.asarray(H, dtype=np.float32))
    adj = np.ascontiguousarray(np.asarray(adj_matrix, dtype=np.float32))
    W = np.asarray(W, dtype=np.float32)
    b = np.asarray(b, dtype=np.float32)

    # Degrees (with self loop), scales
    deg = adj.sum(axis=0, dtype=np.float32) + 1.0
    d = deg.astype(np.float32) ** -0.5
    d = np.where(np.isinf(d), np.float32(0.0), d).astype(np.float32)

    # W folded into H (f32 BLAS), then column scale d; packed k-major:
    # HWP[p, kt*512+q] = (d * (H @ W.T))[kt*128+p, q]
    HsW = d[:, None] * (H @ W.T)
    HWP = (
        HsW.reshape(KT, 128, NOUT).transpose(1, 0, 2).reshape(128, KT * NOUT)
    ).astype(NPBF)

    # S''^T in bf16 via cache-blocked transpose with the row scale d_m
    # folded in, then the normalized self-loop diagonal.
    adjT_bf = np.empty((N, N), dtype=NPBF)
    BLK = 256
    for i in range(0, N, BLK):
        adjT_bf[:, i : i + BLK] = (adj[i : i + BLK, :] * d[i : i + BLK, None]).T.astype(
            NPBF
        )
    idx = np.arange(N)
    adjT_bf[idx, idx] = ((adj[idx, idx] + 1.0) * d).astype(NPBF)

    Bt = np.ascontiguousarray(b.reshape(4, 128).T)

    in_maps = []
    for c in range(NC):
        r0, r1 = c * RB, (c + 1) * RB
        # S pack: [p, kt*1024 + j] = S''[kt*128+p, r0+j]
        X = adjT_bf[:, r0:r1].reshape(KT, 128, RB).transpose(1, 0, 2)
        in_maps.append(
            {
                "S": np.ascontiguousarray(X).reshape(128, KT * RB),
                "HWP": HWP,
                "Bt": Bt,
            }
        )

    if _CACHED_NC is None:
        _CACHED_NC = _build()
    globals()["_LAST_IN_MAPS"] = in_maps
    res = run_bass_kernel_spmd(_CACHED_NC, in_maps, core_ids=list(range(NC)))

    out = np.empty((N, NOUT), dtype=np.float32)
    for c in range(NC):
        r0 = c * RB
        # Out[p, (cc*2+mb)*512+j] = OutT[cc*128+p, mb*512+j] -> rows r0+m
        X = res.results[c]["out"].reshape(128, 4, 2, 512)
        out[r0 : r0 + RB, :] = (
            X.transpose(2, 3, 1, 0).reshape(RB, NOUT).astype(np.float32)
        )
    return out


# revision 17
# speedup vs baseline: 1.0482x; 1.0482x over previous
"""GCN layer on 8 trn2 NeuronCores.

out = tanh( (D^-1/2 (adj+I) D^-1/2) @ H @ W.T + b ), N=8192, nin=nout=512.

Associativity + normalization folding: with d = deg^-0.5,
  out = tanh( S''^T @ HsW + b )  where
  S''[k, m] = d_m * (adj + I)[m, k]   (fully-normalized adjacency, on host)
  HsW[k, :] = d_k * (H @ W.T)[k, :]   (W folded into H on host: one small
                                       4.3-GFLOP BLAS gemm)
so the device runs a SINGLE big gemm (8192x1024x512 per core, bf16) plus a
fused bias+tanh activation per PSUM bank. No second gemm, no transposes, no
PSUM->SBUF copies.

Sharding: output rows (and adj rows) split across 8 cores, 1024 rows each.

Device per core:
  OutT[nout, m] = sum_k HsW[k, nout] * S''[k, m]
    - stationary lhsT = HsW k-chunk [128, 128] (reused for both 512-col
      m-halves), moving rhs = packed S'' strip [128, 512].
    - 4 nout-chunks x 2 m-halves accumulate the full 64-k-tile contraction
      in all 8 PSUM banks simultaneously.
  res = tanh(acc + b_chunk)  (scalar engine, per-partition bias = b slice)
Output lands transposed ([nout, m] blocks); the host transposes it back.

All SWDGE DMAs drain FIFO on one logical queue, so issue order is arrival
order: HsW slices are interleaved with the S strip chunks in exactly the
order the k-loop consumes them, and the first chunks are small to cut
startup latency (first matmul needs only 0.75 MB). The last chunk runs
bank-major so banks stop staggered and the tanh+store tail overlaps the
remaining matmuls. Output stores alternate HWDGE/SWDGE rings.
"""

import sys

sys.path.insert(0, "/opt/trn_rl_repo")

import numpy as np
import ml_dtypes

from concourse import bass, bacc, tile, mybir
from concourse.bass_utils import run_bass_kernel_spmd

N = 8192
NIN = 512
NOUT = 512
NC = 8
RB = N // NC  # 1024 rows per core
KT = N // 128  # 64 k-tiles
CHUNKS = [2, 2, 4] + [8] * 7  # k-tiles per S strip chunk (sum = 64)
F32 = mybir.dt.float32
BF16 = mybir.dt.bfloat16
NPBF = ml_dtypes.bfloat16

_CACHED_NC = None


def _build():
    nc = bacc.Bacc(None, target_bir_lowering=False)

    # Per-core inputs (packed layouts, see kernel() glue)
    S = nc.dram_tensor("S", [128, KT * RB], BF16, kind="ExternalInput")
    HWP = nc.dram_tensor("HWP", [128, KT * NOUT], BF16, kind="ExternalInput")
    Bt = nc.dram_tensor("Bt", [128, 4], F32, kind="ExternalInput")
    # Output transposed: col block (c*2+mb)*512 holds OutT[c-chunk, mb-half]
    Out = nc.dram_tensor("out", [128, 8 * 512], BF16, kind="ExternalOutput")

    with tile.TileContext(nc) as tc:
        with (
            tc.tile_pool(name="persist", bufs=1) as persist,
            tc.tile_pool(name="strip", bufs=6) as striper,
            tc.tile_pool(name="res", bufs=4) as resp,
            tc.tile_pool(name="acc", bufs=2, space=bass.MemorySpace.PSUM) as pacc,
        ):
            # HsW resident: partition p, col kt*512+q holds HsW[kt*128+p, q]
            hw_big = persist.tile([128, KT * NOUT], BF16)
            b_t = persist.tile([128, 4], F32)

            # Both m-halves accumulate across the whole k loop: 8 banks.
            acc0 = pacc.tile([128, 4 * 512], F32, tag="acc")
            acc1 = pacc.tile([128, 4 * 512], F32, tag="acc")
            accs = (acc0, acc1)

            # HAM warm-up: short dummy matmuls on memset scratch keep the PE
            # busy through its ~3.4us activity window while the first strips
            # DMA, so the real gemm starts at 2.4 GHz instead of 1.2. N=128
            # keeps them cheap (~107ns cold each) so the FIFO clears right as
            # the first data lands; the real start=True clears the bank.
            scratch = persist.tile([128, 128], BF16)
            nc.vector.memset(scratch[:], 0.0)
            for _ in range(40):
                nc.tensor.matmul(
                    acc0[:, 0:128], scratch[:], scratch[:], start=True, stop=True
                )

            def mm(kt, c, mb, strip, ktl):
                nc.tensor.matmul(
                    accs[mb][:, c * 512 : (c + 1) * 512],
                    hw_big[:, kt * NOUT + c * 128 : kt * NOUT + (c + 1) * 128],
                    strip[:, ktl * RB + mb * 512 : ktl * RB + (mb + 1) * 512],
                    start=(kt == 0),
                    stop=(kt == KT - 1),
                )

            kt0 = 0
            for ci, cn in enumerate(CHUNKS):
                # Interleave the HsW slice for this k-range ahead of its
                # strip (single SWDGE FIFO: issue order == arrival order).
                hsl = slice(kt0 * NOUT, (kt0 + cn) * NOUT)
                nc.gpsimd.dma_start(hw_big[:, hsl], HWP[:, hsl])
                strip = striper.tile([128, 8 * RB], BF16, tag="strip")
                ssl = slice(kt0 * RB, (kt0 + cn) * RB)
                nc.gpsimd.dma_start(strip[:, 0 : cn * RB], S[:, ssl])
                if ci == 1:
                    nc.gpsimd.dma_start(b_t[:], Bt[:, :])
                last = ci == len(CHUNKS) - 1
                if not last:
                    for ktl in range(cn):
                        for c in range(4):
                            for mb in range(2):
                                mm(kt0 + ktl, c, mb, strip, ktl)
                else:
                    # Bank-major: each bank stops staggered so its tanh +
                    # store overlaps the remaining banks' matmuls.
                    for c in range(4):
                        for mb in range(2):
                            for ktl in range(cn):
                                mm(kt0 + ktl, c, mb, strip, ktl)
                            res = resp.tile([128, 512], BF16, tag="res")
                            nc.scalar.activation(
                                res[:],
                                accs[mb][:, c * 512 : (c + 1) * 512],
                                mybir.ActivationFunctionType.Tanh,
                                bias=b_t[:, c : c + 1],
                            )
                            blk = (c * 2 + mb) * 512
                            eng = nc.gpsimd if (c * 2 + mb) % 2 == 0 else nc.sync
                            eng.dma_start(Out[:, blk : blk + 512], res[:])
                kt0 += cn

    nc.compile()
    return nc


def kernel(H, adj_matrix, W, b):
    global _CACHED_NC
    H = np.ascontiguousarray(np.asarray(H, dtype=np.float32))
    adj = np.ascontiguousarray(np.asarray(adj_matrix, dtype=np.float32))
    W = np.asarray(W, dtype=np.float32)
    b = np.asarray(b, dtype=np.float32)

    # Degrees (with self loop), scales
    deg = adj.sum(axis=0, dtype=np.float32) + 1.0
    d = deg.astype(np.float32) ** -0.5
    d = np.where(np.isinf(d), np.float32(0.0), d).astype(np.float32)

    # W folded into H (f32 BLAS), then column scale d; packed k-major:
    # HWP[p, kt*512+q] = (d * (H @ W.T))[kt*128+p, q]
    HsW = d[:, None] * (H @ W.T)
    HWP = (
        HsW.reshape(KT, 128, NOUT).transpose(1, 0, 2).reshape(128, KT * NOUT)
    ).astype(NPBF)

    # S''^T in bf16 via cache-blocked transpose with the row scale d_m
    # folded in, then the normalized self-loop diagonal.
    adjT_bf = np.empty((N, N), dtype=NPBF)
    BLK = 256
    for i in range(0, N, BLK):
        adjT_bf[:, i : i + BLK] = (adj[i : i + BLK, :] * d[i : i + BLK, None]).T.astype(
            NPBF
        )
    idx = np.arange(N)
    adjT_bf[idx, idx] = ((adj[idx, idx] + 1.0) * d).astype(NPBF)

    Bt = np.ascontiguousarray(b.reshape(4, 128).T)

    in_maps = []
    for c in range(NC):
        r0, r1 = c * RB, (c + 1) * RB
        # S pack: [p, kt*1024 + j] = S''[kt*128+p, r0+j]
        X = adjT_bf[:, r0:r1].reshape(KT, 128, RB).transpose(1, 0, 2)
        in_maps.append(
            {
                "S": np.ascontiguousarray(X).reshape(128, KT * RB),
                "HWP": HWP,
                "Bt": Bt,
            }
        )

    if _CACHED_NC is None:
        _CACHED_NC = _build()
    globals()["_LAST_IN_MAPS"] = in_maps
    res = run_bass_kernel_spmd(_CACHED_NC, in_maps, core_ids=list(range(NC)))

    out = np.empty((N, NOUT), dtype=np.float32)
    for c in range(NC):
        r0 = c * RB
        # Out[p, (cc*2+mb)*512+j] = OutT[cc*128+p, mb*512+j] -> rows r0+m
        X = res.results[c]["out"].reshape(128, 4, 2, 512)
        out[r0 : r0 + RB, :] = (
            X.transpose(2, 3, 1, 0).reshape(RB, NOUT).astype(np.float32)
        )
    return out


# revision 19
# speedup vs baseline: 1.0880x; 1.0380x over previous
"""GCN layer on 8 trn2 NeuronCores.

out = tanh( (D^-1/2 (adj+I) D^-1/2) @ H @ W.T + b ), N=8192, nin=nout=512.

Associativity + normalization folding: with d = deg^-0.5,
  out = tanh( S''^T @ HsW + b )  where
  S''[k, m] = d_m * (adj + I)[m, k]   (fully-normalized adjacency, on host)
  HsW[k, :] = d_k * (H @ W.T)[k, :]   (W folded into H on host: one small
                                       4.3-GFLOP BLAS gemm)
so the device runs a SINGLE big gemm (8192x1024x512 per core, bf16) plus a
fused bias+tanh activation per PSUM bank. No second gemm, no transposes, no
PSUM->SBUF copies.

Sharding: output rows (and adj rows) split across 8 cores, 1024 rows each.

Device per core:
  OutT[nout, m] = sum_k HsW[k, nout] * S''[k, m]
    - stationary lhsT = HsW k-chunk [128, 128] (reused for both 512-col
      m-halves), moving rhs = packed S'' strip [128, 512].
    - 4 nout-chunks x 2 m-halves accumulate the full 64-k-tile contraction
      in all 8 PSUM banks simultaneously.
  res = tanh(acc + b_chunk)  (scalar engine, per-partition bias = b slice)
Output lands transposed ([nout, m] blocks); the host transposes it back.

All SWDGE DMAs drain FIFO on one logical queue, so issue order is arrival
order: HsW slices are interleaved with the S strip chunks in exactly the
order the k-loop consumes them, and the first chunks are small to cut
startup latency (first matmul needs only 0.75 MB). The last chunk runs
bank-major so banks stop staggered and the tanh+store tail overlaps the
remaining matmuls. Output stores alternate HWDGE/SWDGE rings.
"""

import sys

sys.path.insert(0, "/opt/trn_rl_repo")

import numpy as np
import ml_dtypes

from concourse import bass, bacc, tile, mybir
from concourse.bass_utils import run_bass_kernel_spmd

N = 8192
NIN = 512
NOUT = 512
NC = 8
RB = N // NC  # 1024 rows per core
KT = N // 128  # 64 k-tiles
CHUNKS = [2, 2, 4] + [8] * 7  # k-tiles per S strip chunk (sum = 64)
F32 = mybir.dt.float32
BF16 = mybir.dt.bfloat16
NPBF = ml_dtypes.bfloat16

_CACHED_NC = None


def _build():
    nc = bacc.Bacc(None, target_bir_lowering=False)

    # Per-core inputs (packed layouts, see kernel() glue)
    S = nc.dram_tensor("S", [128, KT * RB], BF16, kind="ExternalInput")
    HWP = nc.dram_tensor("HWP", [128, KT * NOUT], BF16, kind="ExternalInput")
    Bt = nc.dram_tensor("Bt", [128, 4], F32, kind="ExternalInput")
    # Output transposed: col block (c*2+mb)*512 holds OutT[c-chunk, mb-half]
    Out = nc.dram_tensor("out", [128, 8 * 512], BF16, kind="ExternalOutput")

    with tile.TileContext(nc) as tc:
        with (
            tc.tile_pool(name="persist", bufs=1) as persist,
            tc.tile_pool(name="strip", bufs=6) as striper,
            tc.tile_pool(name="res", bufs=4) as resp,
            tc.tile_pool(name="acc", bufs=2, space=bass.MemorySpace.PSUM) as pacc,
        ):
            # HsW resident: partition p, col kt*512+q holds HsW[kt*128+p, q]
            hw_big = persist.tile([128, KT * NOUT], BF16)
            b_t = persist.tile([128, 4], F32)

            # Both m-halves accumulate across the whole k loop: 8 banks.
            acc0 = pacc.tile([128, 4 * 512], F32, tag="acc")
            acc1 = pacc.tile([128, 4 * 512], F32, tag="acc")
            accs = (acc0, acc1)

            # HAM warm-up: short dummy matmuls on memset scratch keep the PE
            # busy through its ~3.4us activity window while the first strips
            # DMA, so the real gemm starts at 2.4 GHz instead of 1.2. N=128
            # keeps them cheap (~107ns cold each) so the FIFO clears right as
            # the first data lands; the real start=True clears the bank.
            scratch = persist.tile([128, 128], BF16)
            nc.vector.memset(scratch[:], 0.0)
            for _ in range(56):
                nc.tensor.matmul(
                    acc0[:, 0:128], scratch[:], scratch[:], start=True, stop=True
                )

            def mm(kt, c, mb, strip, ktl):
                nc.tensor.matmul(
                    accs[mb][:, c * 512 : (c + 1) * 512],
                    hw_big[:, kt * NOUT + c * 128 : kt * NOUT + (c + 1) * 128],
                    strip[:, ktl * RB + mb * 512 : ktl * RB + (mb + 1) * 512],
                    start=(kt == 0),
                    stop=(kt == KT - 1),
                )

            kt0 = 0
            for ci, cn in enumerate(CHUNKS):
                # Interleave the HsW slice for this k-range ahead of its
                # strip (single SWDGE FIFO: issue order == arrival order).
                hsl = slice(kt0 * NOUT, (kt0 + cn) * NOUT)
                nc.gpsimd.dma_start(hw_big[:, hsl], HWP[:, hsl])
                strip = striper.tile([128, 8 * RB], BF16, tag="strip")
                # Split strip loads into <=4-kt sub-DMAs: matmul waits are
                # per-DMA-instruction, so finer quanta smooth the ramp.
                for s0 in range(0, cn, 4):
                    sn = min(4, cn - s0)
                    nc.gpsimd.dma_start(
                        strip[:, s0 * RB : (s0 + sn) * RB],
                        S[:, (kt0 + s0) * RB : (kt0 + s0 + sn) * RB],
                    )
                if ci == 1:
                    nc.gpsimd.dma_start(b_t[:], Bt[:, :])
                last = ci == len(CHUNKS) - 1
                if not last:
                    for ktl in range(cn):
                        for c in range(4):
                            for mb in range(2):
                                mm(kt0 + ktl, c, mb, strip, ktl)
                else:
                    # Bank-major: each bank stops staggered so its tanh +
                    # store overlaps the remaining banks' matmuls.
                    for c in range(4):
                        for mb in range(2):
                            for ktl in range(cn):
                                mm(kt0 + ktl, c, mb, strip, ktl)
                            res = resp.tile([128, 512], BF16, tag="res")
                            nc.scalar.activation(
                                res[:],
                                accs[mb][:, c * 512 : (c + 1) * 512],
                                mybir.ActivationFunctionType.Tanh,
                                bias=b_t[:, c : c + 1],
                            )
                            blk = (c * 2 + mb) * 512
                            eng = nc.gpsimd if (c * 2 + mb) % 2 == 0 else nc.sync
                            eng.dma_start(Out[:, blk : blk + 512], res[:])
                kt0 += cn

    nc.compile()
    return nc


def kernel(H, adj_matrix, W, b):
    global _CACHED_NC
    H = np.ascontiguousarray(np.asarray(H, dtype=np.float32))
    adj = np.ascontiguousarray(np.asarray(adj_matrix, dtype=np.float32))
    W = np.asarray(W, dtype=np.float32)
    b = np.asarray(b, dtype=np.float32)

    # Degrees (with self loop), scales
    deg = adj.sum(axis=0, dtype=np.float32) + 1.0
    d = deg.astype(np.float32) ** -0.5
    d = np.where(np.isinf(d), np.float32(0.0), d).astype(np.float32)

    # W folded into H (f32 BLAS), then column scale d; packed k-major:
    # HWP[p, kt*512+q] = (d * (H @ W.T))[kt*128+p, q]
    HsW = d[:, None] * (H @ W.T)
    HWP = (
        HsW.reshape(KT, 128, NOUT).transpose(1, 0, 2).reshape(128, KT * NOUT)
    ).astype(NPBF)

    # S''^T in bf16 via cache-blocked transpose with the row scale d_m
    # folded in, then the normalized self-loop diagonal.
    adjT_bf = np.empty((N, N), dtype=NPBF)
    BLK = 256
    for i in range(0, N, BLK):
        adjT_bf[:, i : i + BLK] = (adj[i : i + BLK, :] * d[i : i + BLK, None]).T.astype(
            NPBF
        )
    idx = np.arange(N)
    adjT_bf[idx, idx] = ((adj[idx, idx] + 1.0) * d).astype(NPBF)

    Bt = np.ascontiguousarray(b.reshape(4, 128).T)

    in_maps = []
    for c in range(NC):
        r0, r1 = c * RB, (c + 1) * RB
        # S pack: [p, kt*1024 + j] = S''[kt*128+p, r0+j]
        X = adjT_bf[:, r0:r1].reshape(KT, 128, RB).transpose(1, 0, 2)
        in_maps.append(
            {
                "S": np.ascontiguousarray(X).reshape(128, KT * RB),
                "HWP": HWP,
                "Bt": Bt,
            }
        )

    if _CACHED_NC is None:
        _CACHED_NC = _build()
    globals()["_LAST_IN_MAPS"] = in_maps
    res = run_bass_kernel_spmd(_CACHED_NC, in_maps, core_ids=list(range(NC)))

    out = np.empty((N, NOUT), dtype=np.float32)
    for c in range(NC):
        r0 = c * RB
        # Out[p, (cc*2+mb)*512+j] = OutT[cc*128+p, mb*512+j] -> rows r0+m
        X = res.results[c]["out"].reshape(128, 4, 2, 512)
        out[r0 : r0 + RB, :] = (
            X.transpose(2, 3, 1, 0).reshape(RB, NOUT).astype(np.float32)
        )
    return out
